# revision 74
# baseline (speedup 1.0000x reference)
"""Trainium2 Bass kernel for nn_AttentionBlock (B=8, C=512, H=W=32, 8 heads).

Sharding: data-parallel over batch — core b computes batch image b end-to-end
(attention is independent per (batch, head); weights replicated to all cores).

Per-core pipeline (x_b viewed as (C=512, S=1024) channels-on-partition):
  P1a: q,k = Wqk^T.T @ x  (f32r)      -> (1024, S), head-pair tile layout
  P1b: vT  = x.T @ Wv^T   (f32r)      -> (S, 512) -> fp8 vta with a ones
       column per head (softmax denominators for free).
  P2 : scoresT[s,t] = k^T q per head (f32r), head pairs row-tiled in the PE.
  exp: ACT exp(s/8 - 1.5) PSUM -> fp8 et tiles shaped (128, 2, 1024): two
       adjacent s-tiles per buffer = the DoubleRow contraction pair.
  P3 : outT_aug[h] = [vT|1]^T @ expT via fp8e4 DoubleRow matmuls (0.5
       cycles/row, 2 s-tiles per instruction): (65, S), row 64 = denom.
  norm: DVE reciprocal -> gpsimd partition_broadcast -> DVE multiply -> fp8
       res (no PSUM/PE broadcast needed).
  P4 : y = Wo8^T.T @ res8 via fp8 DoubleRow + bo + x (stt) -> DMA out.

Scheduling: the emission order software-pipelines the PE stream — P2 for
round r is emitted BEFORE the secondary work (P3/p1a/p1b/norm) of round
r-2, so the next exp's scores are always computed during the current exp
and ACT runs back-to-back.  A warmup burst of junk matmuls ramps the PE
p-state while the input DMAs land.  Tiny "corner" matmuls and scratch
copies act as semaphore-wait carriers because several walrus instruction
structs encode only a single wait (see pe_mm/dve_sync/gp_sync and
_strip_self_waits/_install_drain_split).
"""

import os
import sys

for _p in ("/opt/trn_rl_repo", "/root/.axon_site/_ro/trn_rl_repo"):
    if os.path.isdir(_p) and _p not in sys.path:
        sys.path.insert(0, _p)

from contextlib import ExitStack

import numpy as np

import concourse.bass as bass
import concourse.tile as tile
from concourse import library_config, mybir
from concourse.bass_utils import run_bass_kernel_spmd

B, C, H, W = 8, 512, 32, 32
NH, D = 8, 64
S = H * W            # 1024 sequence positions
P = 128              # partitions
KT = C // P          # 4 contraction tiles over channels
MT_QK = 2 * C // P   # 8 output tiles for q,k
NT = S // P          # 8 t-tiles
NPAIR = NH // 2      # 4 head pairs
NRND = NPAIR * NT    # 32 global rounds
DA = D + 1           # 65: v columns + ones column per head
DP = 2 * D           # 128: per-head vta stride, padded so the DoubleRow
                     # ldweights is the (128, [2, 128]) shape walrus accepts
F32 = mybir.dt.float32
F8 = mybir.dt.float8e4
AF = mybir.ActivationFunctionType
ALU = mybir.AluOpType
MM = mybir.MatmulPerfMode
EXP_BIAS = -1.5      # exp(s/8 - 1.5): keeps fp8e4m3 outputs in (0, 240)

EXP_BUFS = int(os.environ.get("K_EXP_BUFS", "12"))
ACT_K = int(os.environ.get("K_ACT_K", "8"))
NWARM = int(os.environ.get("K_WARM", "8"))
NJUNK = int(os.environ.get("K_JUNK", "24"))
USE_F32R = os.environ.get("K_F32R", "1") == "1"


def _r(ap):
    """Matmul-operand dtype: float32r streams 1 col/cycle (vs 4 for fp32)."""
    if ap.dtype != mybir.dt.float32:
        return ap
    return ap.bitcast(mybir.dt.float32r) if USE_F32R else ap


def _install_drain_split():
    """walrus's CTRL_NO (drain) codegen accepts only a single semaphore wait,
    but Tile's kernel-tail drain aggregates one wait per live proc.  Split
    them across several serial drains (semantically identical: all complete
    before the closing all-engine barrier)."""
    if getattr(tile.TileContext, "_drain_split_installed", False):
        return
    from concourse.vector_clock import ScopedClock

    orig = tile.TileContext._drain_and_barrier

    def patched(self, tick_clock, wait_clock):
        nc = self.nc
        drain_inst = nc.sync.drain()
        wait_clock.add_sem_waits(
            drain_inst.ins, ScopedClock({None: tick_clock.global_clock})
        )
        si = drain_inst.ins.sync_info
        if si is not None and si.on_wait and len(si.on_wait) > 1:
            waits = list(si.on_wait)
            drain_inst.ins.sync_info = mybir.SyncInfo(
                on_wait=[waits[0]], on_update=list(si.on_update or [])
            )
            for w in waits[1:]:
                d2 = nc.sync.drain()
                d2.ins.sync_info = mybir.SyncInfo(on_wait=[w], on_update=[])

        nc.all_engine_barrier()
        assert self.sems is not None
        popped = nc._tile_sem_poison_stack.pop()
        assert popped is self._sem_poison
        nc.clear_and_free_semaphores(list(self.sems.allocated().values()))
        nc.all_engine_barrier()

    tile.TileContext._drain_and_barrier = patched
    tile.TileContext._drain_split_installed = True
    tile.TileContext._drain_and_barrier_orig = orig


def trace_kernel(ctx, tc, nc, x, x8, w8, bo_d, y):
    cst = ctx.enter_context(tc.tile_pool(name="cst", bufs=1))
    qkp = ctx.enter_context(tc.tile_pool(name="qkp", bufs=4))
    expp = ctx.enter_context(tc.tile_pool(name="expp", bufs=EXP_BUFS))
    resp = ctx.enter_context(tc.tile_pool(name="resp", bufs=1))
    rdp = ctx.enter_context(tc.tile_pool(name="rdp", bufs=4))
    rbp = ctx.enter_context(tc.tile_pool(name="rbp", bufs=4))
    yp = ctx.enter_context(tc.tile_pool(name="yp", bufs=1))
    pa = ctx.enter_context(tc.tile_pool(name="pa", bufs=2, space="PSUM"))
    pb = ctx.enter_context(tc.tile_pool(name="pb", bufs=2, space="PSUM"))

    xt = cst.tile([P, KT, S], F32)
    x8t = cst.tile([P, KT, S], F8)
    w8t = cst.tile([P, KT, 4 * C], F8)
    wqk8 = w8t[:, :, 0:2 * C]
    wv8 = w8t[:, :, 2 * C:3 * C]
    wot8 = w8t[:, :, 3 * C:4 * C]
    bot = cst.tile([P, KT, 1], F32)
    ones = cst.tile([1, D], F32)
    scr = cst.tile([1, 256], F32)
    scra = cst.tile([1, 8], F32)
    warm = cst.tile([P, 512], F32)
    vta = cst.tile([P, NT, NH * DP], F8)
    res = resp.tile([P, KT, S], F8)

    nc.gpsimd.memset(warm[:, :], 1.0)

    # DMA order == first-need order (DMA_ENGINES serializes transfers).
    # fp8 inputs are tiny, so the attention pipeline starts ~4x earlier;
    # fp32 x is only needed by the residual add at the very end.
    xr = x.rearrange("(k p) s -> p k s", p=P)
    x8r = x8.rearrange("(k p) s -> p k s", p=P)
    w8r = w8.rearrange("(k p) s -> p k s", p=P)
    bor = bo_d.rearrange("(k p) s -> p k s", p=P)
    nc.sync.dma_start(out=w8t[:, :, 0:256], in_=w8r[:, :, 0:256])
    nc.sync.dma_start(out=x8t[:, :, :], in_=x8r)
    nc.gpsimd.dma_start(out=w8t[:, :, 2 * C:3 * C], in_=w8r[:, :, 2 * C:3 * C])
    nc.gpsimd.dma_start(out=w8t[:, :, 256:2 * C], in_=w8r[:, :, 256:2 * C])
    nc.gpsimd.dma_start(out=w8t[:, :, 3 * C:], in_=w8r[:, :, 3 * C:])
    nc.gpsimd.dma_start(out=bot[:, :, :], in_=bor)
    # fp32 x is tail-only (residual): trigger it LAST on the slow Pool
    # queue so it cannot jump ahead of the fp8 chunks on DMA_ENGINES
    nc.gpsimd.dma_start(out=_r(xt[:, :, :]), in_=_r(xr))

    # fp8 ones in the vta pad region: col 64 per head is the softmax
    # denominator column, cols 65-127 are walrus-shape padding (their oa
    # rows are never read).  Pool memset, after the DMA triggers.
    nc.gpsimd.memset(
        vta.rearrange("p j (h e) -> p j h e", h=NH)[:, :, :, D:DP], 1.0)
    # f32r ones row for the denominator-broadcast outer products
    nc.scalar.activation(_r(ones[:, :]), x8t[0:1, 0, 0:D], AF.Exp, scale=0.0)

    scr_i = [0]

    def dve_sync(*aps):
        # DVE wait-carrier: absorb one cross-engine wait per tiny copy.
        # Disjoint scratch columns avoid WAW self-waits between carriers.
        for ap in aps:
            n = ap.free_size()
            o = (scr_i[0] % 30) * 8
            scr_i[0] += 1
            nc.vector.tensor_copy(scr[0:1, o:o + n], ap)

    def pe_mm(corner, dep):
        # PE wait-carrier: a 1x2 matmul reading `dep` absorbs one cross-
        # engine wait; PE program order subsumes the tick for later matmuls.
        # `corner` is a PSUM slice overwritten by the next start=True group.
        nc.tensor.matmul(
            corner, _r(dep[:, 0:1]), _r(dep[:, 0:2]),
            start=True, stop=True, skip_group_check=True,
        )

    # PSUM: pa's 2 slots rotate sc (score) tiles / vacc / warmup / p4;
    # pb's 2 slots hold the oa accumulators (and p4 accs at the tail).
    def acc_tile(i, shape):
        return pb.tile(shape, F32, tag="ob", name=f"acc{i}")

    qk_tiles = [None] * NPAIR
    nacc = [0]
    ets_hist = []

    def act_sync_maybe():
        # Batched ACT wait-carrier: exp tiles cycle through EXP_BUFS slots;
        # each reuse makes the next exp wait on the slot's previous ACT
        # writer.  One cheap ACT copy pre-waiting on a newer tick covers the
        # next ACT_K reuses (the ACT semaphore is monotonic).
        n = len(ets_hist)
        if n >= EXP_BUFS and (n - EXP_BUFS) % ACT_K == 0:
            nc.scalar.copy(scra[0:1, 0:2],
                           ets_hist[n - EXP_BUFS + ACT_K][0:1, 0, 0:2])

    def p1a_mtile(m, sync_ap=None, w8_corner=None):
        # one full q/k m-tile via fp8 DoubleRow, single eviction.  Aux accs
        # live in the pb (oa) ring, which is free between the previous
        # pair's norm and this pair's first P3 — the sc ring stays pure so
        # the P2->exp stream never waits an aux eviction.
        pair, isk = divmod(m, 2)
        if isk == 0:
            qk_tiles[pair] = qkp.tile([P, 2 * S], F32, tag="qk",
                                      name=f"qk{pair}")
        acc = pb.tile([P, S], F32, tag="ob", name=f"accm{m}")
        if sync_ap is not None:
            pe_mm(acc[0:1, 0:2], sync_ap)
        if w8_corner is not None:
            pe_mm(acc[0:1, 0:2], w8_corner)
        for n in range(2):
            for kk in range(KT // 2):
                nc.tensor.matmul(
                    acc[:, n * 512:(n + 1) * 512],
                    wqk8[:, 2 * kk:2 * kk + 2, m * P:(m + 1) * P],
                    x8t[:, 2 * kk:2 * kk + 2, n * 512:(n + 1) * 512],
                    start=(kk == 0),
                    stop=(kk == KT // 2 - 1),
                    perf_mode=MM.DoubleRow,
                )
        dve_sync(acc[0:1, 252:260])
        nc.vector.tensor_copy(
            _r(qk_tiles[pair][:, isk * S:(isk + 1) * S]), _r(acc[:, :]),
        )

    def p1b_jpair(jj, dve_dep):
        # vT for an s-tile pair through one pb slot, single eviction
        acc = pb.tile([P, S], F32, tag="ob", name=f"vacc{jj}")
        if jj == 0:
            # dve_dep merges with the slot-WAR (same DVE semaphore)
            pe_mm(acc[0:1, 0:2], dve_dep)
            pe_mm(acc[0:1, 0:2], w8t[0:1, 0, 2 * C:2 * C + 2])
        for i in range(2):
            j = 2 * jj + i
            for kk in range(KT // 2):
                nc.tensor.matmul(
                    acc[:, i * 512:(i + 1) * 512],
                    x8t[:, 2 * kk:2 * kk + 2, j * P:(j + 1) * P],
                    wv8[:, 2 * kk:2 * kk + 2, :],
                    start=(kk == 0),
                    stop=(kk == KT // 2 - 1),
                    perf_mode=MM.DoubleRow,
                )
        with nc.allow_low_precision(reason="fp8 vT for DoubleRow attn@v"):
            nc.vector.tensor_copy(
                vta[:, 2 * jj:2 * jj + 2, :]
                .rearrange("p j (h e) -> p j h e", h=NH)[:, :, :, 0:D],
                acc.rearrange("p (j h d) -> p j h d", j=2, h=NH),
            )
        # DVE tick (vta jj) rides on the dead vacc corner
        pe_mm(acc[0:1, 0:2], vta[0:1, 2 * jj, 0:2])

    def p3_dr(pair, jj, oa, ets):
        # fp8 DoubleRow attn@v: the et buffer's two s-tiles contracted per
        # instruction at 0.5 cycles/row
        for hh in range(2):
            h = 2 * pair + hh
            for n in range(2):
                nc.tensor.matmul(
                    oa[hh][:, n * 512:(n + 1) * 512],
                    vta[:, 2 * jj:2 * jj + 2, h * DP:(h + 1) * DP],
                    ets[hh][:, :, n * 512:(n + 1) * 512],
                    start=(jj == 0),
                    stop=(jj == NT // 2 - 1),
                    perf_mode=MM.DoubleRow,
                    skip_group_check=True,
                )

    def norm_hh(pair, hh, oa):
        # res[h] = oa[h][0:64] / oa[h][64]: reciprocal on DVE, ones
        # outer-product broadcast on the PE, evict + multiply on DVE.
        # hh0's broadcast rides a pa (sc-ring) slot; hh1's reuses the
        # PSUM region of oa[hh0], which the hh0 multiply just freed.
        rd = rdp.tile([1, S], F32, tag="rd", name=f"rd{pair}_{hh}")
        with nc.allow_low_precision(reason="f32r view of reciprocal"):
            nc.vector.reciprocal(_r(rd[:, :]), oa[hh][D:DA, :])
        if hh == 0:
            bc = pa.tile([D, S], F32, tag="sc", name=f"bc{pair}_{hh}")
            pe_mm(bc[0:1, 0:2], ets_hist[-1][0:1, 0, 0:2])
            pe_mm(bc[0:1, 0:2], rd[0:1, 0:2])
        else:
            bc = oa[0][0:D, :]
            pe_mm(bc[0:1, 0:2], rd[0:1, 0:2])
        for n in range(2):
            nc.tensor.matmul(
                bc[:, n * 512:(n + 1) * 512],
                _r(ones[:, :]),
                _r(rd[:, n * 512:(n + 1) * 512]),
                start=True,
                stop=True,
                skip_group_check=True,
            )
        rb = rbp.tile([D, S], F32, tag="rb", name=f"rb{pair}_{hh}")
        nc.vector.tensor_copy(rb[:, :], bc[:, :])
        with nc.allow_low_precision(reason="fp8 res for DoubleRow P4"):
            nc.vector.tensor_mul(
                res[64 * hh:64 * (hh + 1), pair, :],
                oa[hh][0:D, :], rb[:, :],
            )

    # ---------------- schedule ----------------
    # Secondary (non-ACT-critical) PE/DVE work lagged two rounds behind the
    # P2->exp stream so the next exp's scores always beat the ACT engine.
    sched = {r: [] for r in range(NRND + 1)}

    def at(r, fn):
        sched[min(r, NRND)].append(fn)

    oa_tiles = [None] * NPAIR
    et_by_jj = {}  # (pair, jj) -> [et_hh0, et_hh1]

    for pair in range(NPAIR):
        base = pair * 8
        # q,k m-tiles for the next pair, in the pb window right after the
        # previous norm vacates it (this pair's rounds 2-3)
        if pair < NPAIR - 1:
            mq, mk = 2 * (pair + 1), 2 * (pair + 1) + 1
            for idx, m_ in enumerate([mq, mk]):
                def mk_p1a(m=m_, first=(idx == 0 and pair == 0)):
                    def go():
                        p1a_mtile(
                            m,
                            sync_ap=(vta[0:1, 0, 0:2] if first else None),
                            w8_corner=(w8t[0:1, 0, 256:258]
                                       if first else None),
                        )
                    return go
                at(base + 2 + idx, mk_p1a())
        # vT s-tile pairs (pair 0 only, pb window before oa is allocated)
        if pair == 0:
            for jj in range(4):
                def mk_p1b(jj=jj):
                    def go():
                        p1b_jpair(jj, qk_tiles[0][0:1, 0:2])
                    return go
                at(jj // 2, mk_p1b())
        # P3 rounds: jj<3 at base+2jj+3; jj3 at base+8 (next pair's r0) for
        # pairs 0-2, immediately in round 31 for the last pair
        for jj in range(4):
            r3 = base + 2 * jj + 3 if jj < 3 else (
                base + 8 if pair < NPAIR - 1 else NRND - 1)
            def mk_p3(pair=pair, jj=jj):
                def go():
                    p3_dr(pair, jj, oa_tiles[pair], et_by_jj[(pair, jj)])
                return go
            at(r3, mk_p3())
        # norm chains: hh0 then hh1 in the two rounds after the last P3
        # (the last pair's norm is emitted latency-interleaved in the tail)
        if pair < NPAIR - 1:
            for hh in range(2):
                def mk_norm(pair=pair, hh=hh):
                    def go():
                        norm_hh(pair, hh, oa_tiles[pair])
                    return go
                at(base + 9 + hh, mk_norm())

    # PE warmup: junk matmuls ramp the p-state while the input DMAs land
    for wi in range(NWARM):
        acc = pa.tile([P, 512], F32, tag="sc", name=f"warm{wi}")
        if wi == 0:
            pe_mm(acc[0:1, 0:2], warm[0:1, 0:2])
        nc.tensor.matmul(acc[:, :], _r(warm[:, 0:128]), _r(warm[:, :]),
                         start=True, stop=True, skip_group_check=True)
    # absorb the vta pad-memset Pool tick so P3 carries a single wait
    pe_mm(acc[0:1, 0:2], vta[0:1, 0, D:D + 2])

    # pair 0 q,k
    p1a_mtile(0, w8_corner=w8t[0:1, 0, 0:2])
    p1a_mtile(1)

    for r in range(NRND):
        pair, j = divmod(r, NT)
        qk = qk_tiles[pair]
        if j % 2 == 0:
            ets = []
            for hh in range(2):
                act_sync_maybe()
                et = expp.tile([P, 2, S], F8, tag="et",
                               name=f"et{pair}_{j // 2}_{hh}")
                ets_hist.append(et)
                ets.append(et)
            et_by_jj[(pair, j // 2)] = ets
        ets = et_by_jj[(pair, j // 2)]
        # P2 + exp for this round (the ACT-critical stream)
        for hh in range(2):
            sc = pa.tile([P, S], F32, tag="sc", name=f"sc{pair}_{j}_{hh}")
            for n in range(2):
                nc.tensor.matmul(
                    sc[:, n * 512:(n + 1) * 512],
                    _r(qk[64 * hh:64 * (hh + 1), S + j * P: S + (j + 1) * P]),
                    _r(qk[64 * hh:64 * (hh + 1), n * 512:(n + 1) * 512]),
                    start=True,
                    stop=True,
                )
            nc.scalar.activation(
                ets[hh][:, j % 2, :], _r(sc[:, :]), AF.Exp,
                scale=1.0 / np.sqrt(D), bias=EXP_BIAS,
            )
        # oa allocation just before this pair's first P3 (round base+3)
        if j == 3:
            oa = [
                pb.tile([P, S], F32, tag="ob", name=f"oa{pair}_{hh}")
                for hh in range(2)
            ]
            oa_tiles[pair] = oa
            if pair == 0:
                pe_mm(oa[0][0:1, 0:2], qk[0:1, S:S + 2])
            else:
                pe_mm(oa[0][0:1, 0:2], res[64:65, pair - 1, 0:2])
        # lagged secondary work
        for fn in sched[r]:
            fn()

    # ---------------- tail ----------------
    # keep the PE p-state hot through the last norm window so the P4
    # DoubleRow matmuls dispatch at full clock
    for wi in range(NJUNK):
        acc = pa.tile([P, 512], F32, tag="sc", name=f"junk{wi}")
        nc.tensor.matmul(acc[:, :], _r(warm[:, 0:128]), _r(warm[:, :]),
                         start=True, stop=True, skip_group_check=True)
    # last pair's norm, latency-interleaved: both reciprocals first (both
    # ready), the PE broadcasts into now-free pa slots, then evict+multiply
    oa3 = oa_tiles[NPAIR - 1]
    t_rds, t_bcs = [], []
    for hh in range(2):
        rd = rdp.tile([1, S], F32, tag="rd", name=f"trd{hh}")
        with nc.allow_low_precision(reason="f32r view of reciprocal"):
            nc.vector.reciprocal(_r(rd[:, :]), oa3[hh][D:DA, :])
        t_rds.append(rd)
    for hh in range(2):
        bc = pa.tile([D, S], F32, tag="sc", name=f"tbc{hh}")
        pe_mm(bc[0:1, 0:2], t_rds[hh][0:1, 0:2])
        for n in range(2):
            nc.tensor.matmul(
                bc[:, n * 512:(n + 1) * 512],
                _r(ones[:, :]),
                _r(t_rds[hh][:, n * 512:(n + 1) * 512]),
                start=True,
                stop=True,
                skip_group_check=True,
            )
        t_bcs.append(bc)
    for hh in range(2):
        rb = rbp.tile([D, S], F32, tag="rb", name=f"trb{hh}")
        nc.vector.tensor_copy(rb[:, :], t_bcs[hh][:, :])
        with nc.allow_low_precision(reason="fp8 res for DoubleRow P4"):
            nc.vector.tensor_mul(
                res[64 * hh:64 * (hh + 1), NPAIR - 1, :],
                oa3[hh][0:D, :], rb[:, :],
            )

    # tail evictions: y = acc + bo + x on DVE (GPSIMD cannot read PSUM on
    # hardware); y chunks stream out on SP-triggered DMAs per m-tile
    dve_sync(xt[0:1, 0, 4:8], bot[0:1, 0, 0:1])
    ybig = yp.tile([P, KT, S], F32, tag="y", name="yb")
    yr = y.rearrange("(k p) s -> p k s", p=P)
    for m in range(KT):
        if m >= 2:
            acc = pa.tile([P, S], F32, tag="sc", name=f"p4acc{m}")
        else:
            acc = acc_tile(m, [P, S])
        if m == 0:
            pe_mm(acc[0:1, 0:2], res[64:65, NPAIR - 1, 0:2])
            pe_mm(acc[0:1, 0:2], wot8[0:1, 0, 0:2])
        elif m == 1:
            pe_mm(acc[0:1, 0:2], res[0:1, NPAIR - 1, 0:2])
        for n in range(2):
            for kk in range(KT // 2):
                nc.tensor.matmul(
                    acc[:, n * 512:(n + 1) * 512],
                    wot8[:, 2 * kk:2 * kk + 2, m * P:(m + 1) * P],
                    res[:, 2 * kk:2 * kk + 2, n * 512:(n + 1) * 512],
                    start=(kk == 0),
                    stop=(kk == KT // 2 - 1),
                    perf_mode=MM.DoubleRow,
                )
        dve_sync(acc[0:1, 508:516])
        nc.vector.scalar_tensor_tensor(
            _r(ybig[:, m, :]), acc[:, :], bot[:, m, :],
            xt[:, m, :], op0=ALU.add, op1=ALU.add,
        )
        nc.sync.dma_start(out=yr[:, m:m + 1, :], in_=ybig[:, m:m + 1, :])


ENGINE_SEM_PREFIX = {
    "PE": "PE_",
    "Activation": "Activation_",
    "DVE": "DVE_",
    "Pool": "Pool_",
    "SP": "SP_",
}


def _strip_self_waits(nc):
    """Drop same-engine semaphore self-waits from multi-wait instructions.

    Engines execute and complete their own instructions in program order
    (PE matmuls are pc-monotone in start and end; ACT/DVE/Pool are strict
    FIFO with per-op drains), so a wait on the engine's own completion
    semaphore is redundant whenever the instruction carries another wait —
    and walrus's PE/ACT instruction structs only encode a single wait.
    """
    n = 0
    for inst in nc.inst_map.values():
        si = getattr(inst, "sync_info", None)
        if si is None or not si.on_wait or len(si.on_wait) <= 1:
            continue
        eng = str(getattr(inst, "engine", "")).split(".")[-1]
        pref = ENGINE_SEM_PREFIX.get(eng)
        if pref is None:
            continue
        keep = [w for w in si.on_wait if not w.ant_name.startswith(pref)]
        if len(keep) != len(si.on_wait) and keep:
            inst.sync_info = mybir.SyncInfo(
                on_wait=keep, on_update=list(si.on_update or [])
            )
            n += 1
    return n


def build_nc():
    _install_drain_split()
    nc = bass.Bass(trn_type="TRN2", debug=False, num_devices=8)
    # pre-register the exp bias as a const AP (like the 0.0/1.0 preamble
    # consts) so the exps' float bias arg resolves without any dep/wait
    _bt = nc.alloc_sbuf_tensor("const-float32-expbias", [128, 1], F32)
    nc.gpsimd.memset(_bt.ap(), EXP_BIAS)
    nc.const_aps.aps[(F32, EXP_BIAS)] = _bt.ap()
    nc.all_engine_barrier()
    x_d = nc.dram_tensor("x", [C, S], F32, kind="ExternalInput")
    x8_d = nc.dram_tensor("x8", [C, S], F8, kind="ExternalInput")
    w8_d = nc.dram_tensor("w8", [C, 4 * C], F8, kind="ExternalInput")
    bo_d = nc.dram_tensor("bo", [C, 1], F32, kind="ExternalInput")
    y_d = nc.dram_tensor("y", [C, S], F32, kind="ExternalOutput")
    with tile.TileContext(nc) as tc, ExitStack() as ctx:
        trace_kernel(ctx, tc, nc, x_d.ap(), x8_d.ap(), w8_d.ap(), bo_d.ap(),
                     y_d.ap())
    _strip_self_waits(nc)
    if not nc.is_finalized():
        nc.finalize()
    return nc


def host_inputs(x, Wqkv, Wo, bo):
    """Host-side reshard: per-core input dicts (weights replicated)."""
    x = np.ascontiguousarray(np.asarray(x, dtype=np.float32))
    Wqkv = np.asarray(Wqkv, dtype=np.float32)
    Wo = np.asarray(Wo, dtype=np.float32)
    bo = np.asarray(bo, dtype=np.float32)

    # Wqkv rows per head h: [h*3D, h*3D+D) = q, [+D, +2D) = k, [+2D, +3D) = v.
    # q,k channel order: per pair -> [q(2p)|q(2p+1)], [k(2p)|k(2p+1)] tiles.
    order = []
    for p in range(NPAIR):
        for h in (2 * p, 2 * p + 1):
            order.extend(range(h * 3 * D, h * 3 * D + D))          # q rows
        for h in (2 * p, 2 * p + 1):
            order.extend(range(h * 3 * D + D, h * 3 * D + 2 * D))  # k rows
    wqkt = Wqkv[order].T                                            # (C, 2C)
    v_order = [h * 3 * D + 2 * D + d for h in range(NH) for d in range(D)]
    wvt = Wqkv[v_order].T                                           # (C, C)
    import ml_dtypes
    w8 = np.ascontiguousarray(
        np.concatenate([wqkt, wvt, Wo.T], axis=1)
    ).astype(ml_dtypes.float8_e4m3)                                 # (C, 4C)
    bo2 = np.ascontiguousarray(bo[:, None])                         # (C, 1)

    return [
        dict(x=np.ascontiguousarray(x[b].reshape(C, S)),
             x8=np.ascontiguousarray(x[b].reshape(C, S)).astype(
                 ml_dtypes.float8_e4m3),
             w8=w8, bo=bo2)
        for b in range(B)
    ]


_NC_CACHE = []

try:
    # bass_exec HLO does not embed the BIR; bust jax's executable cache so a
    # rebuilt kernel is actually recompiled instead of hitting a stale NEFF.
    import jax as _jax

    _jax.clear_caches()
except Exception:
    pass


def get_nc():
    if not _NC_CACHE:
        _NC_CACHE.append(build_nc())
    return _NC_CACHE[0]


def run(in_maps, **kwargs):
    return run_bass_kernel_spmd(get_nc(), in_maps, core_ids=list(range(B)), **kwargs)


def kernel(x, Wqkv, Wo, bo):
    in_maps = host_inputs(x, Wqkv, Wo, bo)
    r = run(in_maps)
    y = np.stack([r.results[b]["y"].reshape(C, H, W) for b in range(B)])
    return y.astype(np.float32)


if __name__ == "__main__":
    nc = build_nc()
    print("built ok:", len(nc.inst_map), "instructions")


# revision 96
# speedup vs baseline: 1.0123x; 1.0123x over previous
"""Trainium2 Bass kernel for nn_AttentionBlock (B=8, C=512, H=W=32, 8 heads).

Sharding: data-parallel over batch — core b computes batch image b end-to-end
(attention is independent per (batch, head); weights replicated to all cores).

Per-core pipeline (x_b viewed as (C=512, S=1024) channels-on-partition):
  P1a: q,k = Wqk^T.T @ x  (f32r)      -> (1024, S), head-pair tile layout
  P1b: vT  = x.T @ Wv^T   (f32r)      -> (S, 512) -> fp8 vta with a ones
       column per head (softmax denominators for free).
  P2 : scoresT[s,t] = k^T q per head (f32r), head pairs row-tiled in the PE.
  exp: ACT exp(s/8 - 1.5) PSUM -> fp8 et tiles shaped (128, 2, 1024): two
       adjacent s-tiles per buffer = the DoubleRow contraction pair.
  P3 : outT_aug[h] = [vT|1]^T @ expT via fp8e4 DoubleRow matmuls (0.5
       cycles/row, 2 s-tiles per instruction): (65, S), row 64 = denom.
  norm: DVE reciprocal -> gpsimd partition_broadcast -> DVE multiply -> fp8
       res (no PSUM/PE broadcast needed).
  P4 : y = Wo8^T.T @ res8 via fp8 DoubleRow + bo + x (stt) -> DMA out.

Scheduling: the emission order software-pipelines the PE stream — P2 for
round r is emitted BEFORE the secondary work (P3/p1a/p1b/norm) of round
r-2, so the next exp's scores are always computed during the current exp
and ACT runs back-to-back.  A warmup burst of junk matmuls ramps the PE
p-state while the input DMAs land.  Tiny "corner" matmuls and scratch
copies act as semaphore-wait carriers because several walrus instruction
structs encode only a single wait (see pe_mm/dve_sync/gp_sync and
_strip_self_waits/_install_drain_split).
"""

import os
import sys

for _p in ("/opt/trn_rl_repo", "/root/.axon_site/_ro/trn_rl_repo"):
    if os.path.isdir(_p) and _p not in sys.path:
        sys.path.insert(0, _p)

from contextlib import ExitStack

import numpy as np

import concourse.bass as bass
import concourse.tile as tile
from concourse import library_config, mybir
from concourse.bass_utils import run_bass_kernel_spmd

B, C, H, W = 8, 512, 32, 32
NH, D = 8, 64
S = H * W            # 1024 sequence positions
P = 128              # partitions
KT = C // P          # 4 contraction tiles over channels
MT_QK = 2 * C // P   # 8 output tiles for q,k
NT = S // P          # 8 t-tiles
NPAIR = NH // 2      # 4 head pairs
NRND = NPAIR * NT    # 32 global rounds
DA = D + 1           # 65: v columns + ones column per head
DP = 2 * D           # 128: per-head vta stride, padded so the DoubleRow
                     # ldweights is the (128, [2, 128]) shape walrus accepts
F32 = mybir.dt.float32
F8 = mybir.dt.float8e4
AF = mybir.ActivationFunctionType
ALU = mybir.AluOpType
MM = mybir.MatmulPerfMode
EXP_BIAS = -1.5      # exp(s/8 - 1.5): keeps fp8e4m3 outputs in (0, 240)

EXP_BUFS = int(os.environ.get("K_EXP_BUFS", "12"))
ACT_K = int(os.environ.get("K_ACT_K", "8"))
NWARM = int(os.environ.get("K_WARM", "8"))
NJUNK = int(os.environ.get("K_JUNK", "12"))
USE_F32R = os.environ.get("K_F32R", "1") == "1"


def _r(ap):
    """Matmul-operand dtype: float32r streams 1 col/cycle (vs 4 for fp32)."""
    if ap.dtype != mybir.dt.float32:
        return ap
    return ap.bitcast(mybir.dt.float32r) if USE_F32R else ap


def _install_drain_split():
    """walrus's CTRL_NO (drain) codegen accepts only a single semaphore wait,
    but Tile's kernel-tail drain aggregates one wait per live proc.  Split
    them across several serial drains (semantically identical: all complete
    before the closing all-engine barrier)."""
    if getattr(tile.TileContext, "_drain_split_installed", False):
        return
    from concourse.vector_clock import ScopedClock

    orig = tile.TileContext._drain_and_barrier

    def patched(self, tick_clock, wait_clock):
        nc = self.nc
        drain_inst = nc.sync.drain()
        wait_clock.add_sem_waits(
            drain_inst.ins, ScopedClock({None: tick_clock.global_clock})
        )
        si = drain_inst.ins.sync_info
        if si is not None and si.on_wait and len(si.on_wait) > 1:
            waits = list(si.on_wait)
            drain_inst.ins.sync_info = mybir.SyncInfo(
                on_wait=[waits[0]], on_update=list(si.on_update or [])
            )
            for w in waits[1:]:
                d2 = nc.sync.drain()
                d2.ins.sync_info = mybir.SyncInfo(on_wait=[w], on_update=[])

        nc.all_engine_barrier()
        assert self.sems is not None
        popped = nc._tile_sem_poison_stack.pop()
        assert popped is self._sem_poison
        nc.clear_and_free_semaphores(list(self.sems.allocated().values()))
        nc.all_engine_barrier()

    tile.TileContext._drain_and_barrier = patched
    tile.TileContext._drain_split_installed = True
    tile.TileContext._drain_and_barrier_orig = orig


def trace_kernel(ctx, tc, nc, x, x8, w8, bo_d, y):
    cst = ctx.enter_context(tc.tile_pool(name="cst", bufs=1))
    qkp = ctx.enter_context(tc.tile_pool(name="qkp", bufs=4))
    expp = ctx.enter_context(tc.tile_pool(name="expp", bufs=EXP_BUFS))
    resp = ctx.enter_context(tc.tile_pool(name="resp", bufs=1))
    rdp = ctx.enter_context(tc.tile_pool(name="rdp", bufs=4))
    rbp = ctx.enter_context(tc.tile_pool(name="rbp", bufs=4))
    yp = ctx.enter_context(tc.tile_pool(name="yp", bufs=1))
    pa = ctx.enter_context(tc.tile_pool(name="pa", bufs=2, space="PSUM"))
    pb = ctx.enter_context(tc.tile_pool(name="pb", bufs=2, space="PSUM"))

    xt = cst.tile([P, KT, S], F32)
    x8t = cst.tile([P, KT, S], F8)
    w8t = cst.tile([P, KT, 4 * C], F8)
    wqk8 = w8t[:, :, 0:2 * C]
    wv8 = w8t[:, :, 2 * C:3 * C]
    wot8 = w8t[:, :, 3 * C:4 * C]
    bot = cst.tile([P, KT, 1 + P], F32)
    ident = bot[:, 0, 1:1 + P]
    ones = cst.tile([1, D], F32)
    scr = cst.tile([1, 256], F32)
    scra = cst.tile([1, 8], F32)
    warm = cst.tile([P, 512], F32)
    vta = cst.tile([P, NT, NH * DP], F8)
    res = resp.tile([P, KT, S], F8)

    nc.gpsimd.memset(warm[:, :], 1.0)

    # DMA order == first-need order (DMA_ENGINES serializes transfers).
    # fp8 inputs are tiny, so the attention pipeline starts ~4x earlier;
    # fp32 x is only needed by the residual add at the very end.
    xr = x.rearrange("(k p) s -> p k s", p=P)
    x8r = x8.rearrange("(k p) s -> p k s", p=P)
    w8r = w8.rearrange("(k p) s -> p k s", p=P)
    bor = bo_d.rearrange("(k p) s -> p k s", p=P)
    # Queue budget: each DGE queue has ~3 credit-free triggers; DMAs that
    # carry a DATA wait (the y chunks) must use those slots, so the inputs
    # (wait-free; a credit wait alone is fine) ride the Pool queue.
    nc.sync.dma_start(out=w8t[:, :, 0:128], in_=w8r[:, :, 0:128])
    nc.sync.dma_start(out=x8t[:, :, :], in_=x8r)
    nc.sync.dma_start(out=w8t[:, :, 128:256], in_=w8r[:, :, 128:256])
    nc.gpsimd.dma_start(out=w8t[:, :, 256:], in_=w8r[:, :, 256:])
    nc.gpsimd.dma_start(out=_r(bot[:, :, :]), in_=_r(bor))
    # fp32 x is tail-only (residual): trigger it LAST on the slow Pool
    # queue so it cannot jump ahead of the fp8 chunks on DMA_ENGINES
    nc.gpsimd.dma_start(out=_r(xt[:, :, :]), in_=_r(xr))

    # fp8 ones in the vta pad region: col 64 per head is the softmax
    # denominator column, cols 65-127 are walrus-shape padding (their oa
    # rows are never read).  Pool memset, after the DMA triggers.
    nc.gpsimd.memset(
        vta.rearrange("p j (h e) -> p j h e", h=NH)[:, :, :, D:DP], 1.0)
    # f32r ones row for the denominator-broadcast outer products
    nc.scalar.activation(_r(ones[:, :]), x8t[0:1, 0, 0:D], AF.Exp, scale=0.0)

    scr_i = [0]

    def dve_sync(*aps):
        # DVE wait-carrier: absorb one cross-engine wait per tiny copy.
        # Disjoint scratch columns avoid WAW self-waits between carriers.
        for ap in aps:
            n = ap.free_size()
            o = (scr_i[0] % 30) * 8
            scr_i[0] += 1
            nc.vector.tensor_copy(scr[0:1, o:o + n], ap)

    def pe_mm(corner, dep):
        # PE wait-carrier: a 1x2 matmul reading `dep` absorbs one cross-
        # engine wait; PE program order subsumes the tick for later matmuls.
        # `corner` is a PSUM slice overwritten by the next start=True group.
        nc.tensor.matmul(
            corner, _r(dep[:, 0:1]), _r(dep[:, 0:2]),
            start=True, stop=True, skip_group_check=True,
        )

    # PSUM: pa's 2 slots rotate sc (score) tiles / vacc / warmup / p4;
    # pb's 2 slots hold the oa accumulators (and p4 accs at the tail).
    def acc_tile(i, shape):
        return pb.tile(shape, F32, tag="ob", name=f"acc{i}")

    qk_tiles = [None] * NPAIR
    nacc = [0]
    ets_hist = []

    def act_sync_maybe():
        # Batched ACT wait-carrier: exp tiles cycle through EXP_BUFS slots;
        # each reuse makes the next exp wait on the slot's previous ACT
        # writer.  One cheap ACT copy pre-waiting on a newer tick covers the
        # next ACT_K reuses (the ACT semaphore is monotonic).
        n = len(ets_hist)
        if n >= EXP_BUFS and (n - EXP_BUFS) % ACT_K == 0:
            nc.scalar.copy(scra[0:1, 0:2],
                           ets_hist[n - EXP_BUFS + ACT_K][0:1, 0, 0:2])

    def p1a_mtile(m, sync_ap=None, w8_corner=None):
        # one full q/k m-tile via fp8 DoubleRow, single eviction.  Aux accs
        # live in the pb (oa) ring, which is free between the previous
        # pair's norm and this pair's first P3 — the sc ring stays pure so
        # the P2->exp stream never waits an aux eviction.
        pair, isk = divmod(m, 2)
        if isk == 0:
            qk_tiles[pair] = qkp.tile([P, 2 * S], F32, tag="qk",
                                      name=f"qk{pair}")
        acc = pb.tile([P, S], F32, tag="ob", name=f"accm{m}")
        if sync_ap is not None:
            pe_mm(acc[0:1, 0:2], sync_ap)
        if w8_corner is not None:
            pe_mm(acc[0:1, 0:2], w8_corner)
        for n in range(2):
            for kk in range(KT // 2):
                nc.tensor.matmul(
                    acc[:, n * 512:(n + 1) * 512],
                    wqk8[:, 2 * kk:2 * kk + 2, m * P:(m + 1) * P],
                    x8t[:, 2 * kk:2 * kk + 2, n * 512:(n + 1) * 512],
                    start=(kk == 0),
                    stop=(kk == KT // 2 - 1),
                    perf_mode=MM.DoubleRow,
                )
        dve_sync(acc[0:1, 252:260])
        nc.vector.tensor_copy(
            _r(qk_tiles[pair][:, isk * S:(isk + 1) * S]), _r(acc[:, :]),
        )

    def p1b_jpair(jj, dve_dep):
        # vT for an s-tile pair through one pb slot, single eviction
        acc = pb.tile([P, S], F32, tag="ob", name=f"vacc{jj}")
        if jj == 0:
            # dve_dep merges with the slot-WAR (same DVE semaphore)
            pe_mm(acc[0:1, 0:2], dve_dep)
            pe_mm(acc[0:1, 0:2], w8t[0:1, 0, 2 * C:2 * C + 2])
        for i in range(2):
            j = 2 * jj + i
            for kk in range(KT // 2):
                nc.tensor.matmul(
                    acc[:, i * 512:(i + 1) * 512],
                    x8t[:, 2 * kk:2 * kk + 2, j * P:(j + 1) * P],
                    wv8[:, 2 * kk:2 * kk + 2, :],
                    start=(kk == 0),
                    stop=(kk == KT // 2 - 1),
                    perf_mode=MM.DoubleRow,
                )
        with nc.allow_low_precision(reason="fp8 vT for DoubleRow attn@v"):
            nc.vector.tensor_copy(
                vta[:, 2 * jj:2 * jj + 2, :]
                .rearrange("p j (h e) -> p j h e", h=NH)[:, :, :, 0:D],
                acc.rearrange("p (j h d) -> p j h d", j=2, h=NH),
            )
        # DVE tick (vta jj) rides on the dead vacc corner
        pe_mm(acc[0:1, 0:2], vta[0:1, 2 * jj, 0:2])

    def p3_dr(pair, jj, oa, ets):
        # fp8 DoubleRow attn@v: the et buffer's two s-tiles contracted per
        # instruction at 0.5 cycles/row
        for hh in range(2):
            h = 2 * pair + hh
            for n in range(2):
                nc.tensor.matmul(
                    oa[hh][:, n * 512:(n + 1) * 512],
                    vta[:, 2 * jj:2 * jj + 2, h * DP:(h + 1) * DP],
                    ets[hh][:, :, n * 512:(n + 1) * 512],
                    start=(jj == 0),
                    stop=(jj == NT // 2 - 1),
                    perf_mode=MM.DoubleRow,
                    skip_group_check=True,
                )

    def norm_hh(pair, hh, oa):
        # res[h] = oa[h][0:64] / oa[h][64]: reciprocal on DVE, ones
        # outer-product broadcast on the PE, evict + multiply on DVE.
        # hh0's broadcast rides a pa (sc-ring) slot; hh1's reuses the
        # PSUM region of oa[hh0], which the hh0 multiply just freed.
        rd = rdp.tile([1, S], F32, tag="rd", name=f"rd{pair}_{hh}")
        with nc.allow_low_precision(reason="f32r view of reciprocal"):
            nc.vector.reciprocal(_r(rd[:, :]), oa[hh][D:DA, :])
        if hh == 0:
            bc = pa.tile([D, S], F32, tag="sc", name=f"bc{pair}_{hh}")
            pe_mm(bc[0:1, 0:2], ets_hist[-1][0:1, 0, 0:2])
            pe_mm(bc[0:1, 0:2], rd[0:1, 0:2])
        else:
            bc = oa[0][0:D, :]
            pe_mm(bc[0:1, 0:2], rd[0:1, 0:2])
        for n in range(2):
            nc.tensor.matmul(
                bc[:, n * 512:(n + 1) * 512],
                _r(ones[:, :]),
                _r(rd[:, n * 512:(n + 1) * 512]),
                start=True,
                stop=True,
                skip_group_check=True,
            )
        rb = rbp.tile([D, S], F32, tag="rb", name=f"rb{pair}_{hh}")
        nc.vector.tensor_copy(rb[:, :], bc[:, :])
        with nc.allow_low_precision(reason="fp8 res for DoubleRow P4"):
            nc.vector.tensor_mul(
                res[64 * hh:64 * (hh + 1), pair, :],
                oa[hh][0:D, :], rb[:, :],
            )

    # ---------------- schedule ----------------
    # Secondary (non-ACT-critical) PE/DVE work lagged two rounds behind the
    # P2->exp stream so the next exp's scores always beat the ACT engine.
    sched = {r: [] for r in range(NRND + 1)}

    def at(r, fn):
        sched[min(r, NRND)].append(fn)

    oa_tiles = [None] * NPAIR
    et_by_jj = {}  # (pair, jj) -> [et_hh0, et_hh1]

    for pair in range(NPAIR):
        base = pair * 8
        # q,k m-tiles for the next pair, in the pb window right after the
        # previous norm vacates it (this pair's rounds 2-3)
        if pair < NPAIR - 1:
            mq, mk = 2 * (pair + 1), 2 * (pair + 1) + 1
            for idx, m_ in enumerate([mq, mk]):
                def mk_p1a(m=m_, first=(idx == 0 and pair == 0)):
                    def go():
                        p1a_mtile(
                            m,
                            sync_ap=(vta[0:1, 0, 0:2] if first else None),
                            w8_corner=(w8t[0:1, 0, 256:258]
                                       if first else None),
                        )
                    return go
                at(base + 2 + idx, mk_p1a())
        # vT s-tile pairs (pair 0 only, pb window before oa is allocated)
        if pair == 0:
            for jj in range(4):
                def mk_p1b(jj=jj):
                    def go():
                        p1b_jpair(jj, qk_tiles[0][0:1, 0:2])
                    return go
                at(jj // 2, mk_p1b())
        # P3 rounds: jj<3 at base+2jj+3; jj3 at base+8 (next pair's r0) for
        # pairs 0-2, immediately in round 31 for the last pair
        for jj in range(4):
            r3 = base + 2 * jj + 3 if jj < 3 else (
                base + 8 if pair < NPAIR - 1 else NRND - 1)
            def mk_p3(pair=pair, jj=jj):
                def go():
                    p3_dr(pair, jj, oa_tiles[pair], et_by_jj[(pair, jj)])
                return go
            at(r3, mk_p3())
        # norm chains: hh0 then hh1 in the two rounds after the last P3
        # (the last pair's norm is emitted latency-interleaved in the tail)
        if pair < NPAIR - 1:
            for hh in range(2):
                def mk_norm(pair=pair, hh=hh):
                    def go():
                        norm_hh(pair, hh, oa_tiles[pair])
                    return go
                at(base + 9 + hh, mk_norm())

    # PE warmup: junk matmuls ramp the p-state while the input DMAs land.
    # Full-slot-sized tiles keep the pa byte-ring aligned for the sc tiles.
    for wi in range(NWARM):
        acc = pa.tile([P, S], F32, tag="sc", name=f"warm{wi}")
        if wi == 0:
            pe_mm(acc[0:1, 0:2], warm[0:1, 0:2])
        nc.tensor.matmul(acc[:, 0:512], _r(warm[:, 0:128]), _r(warm[:, :]),
                         start=True, stop=True, skip_group_check=True)
    # absorb the vta pad-memset Pool tick so P3 carries a single wait
    pe_mm(acc[0:1, 0:2], vta[0:1, 0, D:D + 2])

    # pair 0 q,k
    p1a_mtile(0, w8_corner=w8t[0:1, 0, 0:2])
    p1a_mtile(1)

    for r in range(NRND):
        pair, j = divmod(r, NT)
        qk = qk_tiles[pair]
        if j % 2 == 0:
            ets = []
            for hh in range(2):
                act_sync_maybe()
                et = expp.tile([P, 2, S], F8, tag="et",
                               name=f"et{pair}_{j // 2}_{hh}")
                ets_hist.append(et)
                ets.append(et)
            et_by_jj[(pair, j // 2)] = ets
        ets = et_by_jj[(pair, j // 2)]
        # P2 + exp for this round (the ACT-critical stream)
        for hh in range(2):
            sc = pa.tile([P, S], F32, tag="sc", name=f"sc{pair}_{j}_{hh}")
            for n in range(2):
                nc.tensor.matmul(
                    sc[:, n * 512:(n + 1) * 512],
                    _r(qk[64 * hh:64 * (hh + 1), S + j * P: S + (j + 1) * P]),
                    _r(qk[64 * hh:64 * (hh + 1), n * 512:(n + 1) * 512]),
                    start=True,
                    stop=True,
                )
            nc.scalar.activation(
                ets[hh][:, j % 2, :], _r(sc[:, :]), AF.Exp,
                scale=1.0 / np.sqrt(D), bias=EXP_BIAS,
            )
        # oa allocation just before this pair's first P3 (round base+3)
        if j == 3:
            oa = [
                pb.tile([P, S], F32, tag="ob", name=f"oa{pair}_{hh}")
                for hh in range(2)
            ]
            oa_tiles[pair] = oa
            if pair == 0:
                pe_mm(oa[0][0:1, 0:2], qk[0:1, S:S + 2])
            else:
                pe_mm(oa[0][0:1, 0:2], res[64:65, pair - 1, 0:2])
        # lagged secondary work
        for fn in sched[r]:
            fn()

    # ---------------- tail ----------------
    # last pair's norm, latency-interleaved: both reciprocals first (both
    # ready), PE broadcasts immediately after (before the junk filler),
    # bc evictions on the now-idle ACT, multiplies on DVE
    oa3 = oa_tiles[NPAIR - 1]
    t_rds, t_bcs = [], []
    for hh in range(2):
        rd = rdp.tile([1, S], F32, tag="rd", name=f"trd{hh}")
        with nc.allow_low_precision(reason="f32r view of reciprocal"):
            nc.vector.reciprocal(_r(rd[:, :]), oa3[hh][D:DA, :])
        t_rds.append(rd)
    for hh in range(2):
        bc = pa.tile([D, S], F32, tag="sc", name=f"tbc{hh}")
        pe_mm(bc[0:1, 0:2], t_rds[hh][0:1, 0:2])
        for n in range(2):
            nc.tensor.matmul(
                bc[:, n * 512:(n + 1) * 512],
                _r(ones[:, :]),
                _r(t_rds[hh][:, n * 512:(n + 1) * 512]),
                start=True,
                stop=True,
                skip_group_check=True,
            )
        t_bcs.append(bc)
    # keep the PE p-state hot through the norm window so the P4 DoubleRow
    # matmuls dispatch at full clock
    junk_acc = None
    for wi in range(NJUNK):
        junk_acc = pa.tile([P, S], F32, tag="sc", name=f"junk{wi}")
        nc.tensor.matmul(junk_acc[:, 0:512], _r(warm[:, 0:128]),
                         _r(warm[:, :]),
                         start=True, stop=True, skip_group_check=True)
    for hh in range(2):
        # fresh tiles: a pool slot here would add a WAR wait the walrus
        # single-wait ACT struct cannot encode
        rb = cst.tile([D, S], F32, name=f"trb{hh}")
        nc.scalar.copy(rb[:, :], t_bcs[hh][:, :])
        dve_sync(rb[0:1, 0:2])
        with nc.allow_low_precision(reason="fp8 res for DoubleRow P4"):
            nc.vector.tensor_mul(
                res[64 * hh:64 * (hh + 1), NPAIR - 1, :],
                oa3[hh][0:D, :], rb[:, :],
            )

    # P4 + eviction: y = acc + bo + x on DVE (stt), y DMAs in m-tile pairs
    # on the gpsimd SWDGE queue (the baseline-proven pattern: these DMAs
    # carry a DVE data wait and must stay within the queue credit window)
    dve_sync(xt[0:1, 0, 4:8], bot[0:1, 0, 0:1])
    ybig = yp.tile([P, KT, S], F32, tag="y", name="yb")
    yr = y.rearrange("(k p) s -> p k s", p=P)
    for m in range(KT):
        if m >= 2:
            acc = pa.tile([P, S], F32, tag="sc", name=f"p4acc{m}")
        else:
            acc = acc_tile(m, [P, S])
        if m == 0:
            pe_mm(acc[0:1, 0:2], res[64:65, NPAIR - 1, 0:2])
            pe_mm(acc[0:1, 0:2], wot8[0:1, 0, 0:2])
        elif m == 1:
            pe_mm(acc[0:1, 0:2], res[0:1, NPAIR - 1, 0:2])
        for n in range(2):
            for kk in range(KT // 2):
                nc.tensor.matmul(
                    acc[:, n * 512:(n + 1) * 512],
                    wot8[:, 2 * kk:2 * kk + 2, m * P:(m + 1) * P],
                    res[:, 2 * kk:2 * kk + 2, n * 512:(n + 1) * 512],
                    start=(kk == 0),
                    stop=(kk == KT // 2 - 1),
                    perf_mode=MM.DoubleRow,
                )
        dve_sync(acc[0:1, 508:516])
        nc.vector.scalar_tensor_tensor(
            _r(ybig[:, m, :]), acc[:, :], bot[:, m, 0:1],
            xt[:, m, :], op0=ALU.add, op1=ALU.add,
        )
        if m % 2 == 1:
            nc.gpsimd.dma_start(out=yr[:, m - 1:m + 1, :],
                                in_=ybig[:, m - 1:m + 1, :])


ENGINE_SEM_PREFIX = {
    "PE": "PE_",
    "Activation": "Activation_",
    "DVE": "DVE_",
    "Pool": "Pool_",
    "SP": "SP_",
}


def _strip_self_waits(nc):
    """Drop same-engine semaphore self-waits from multi-wait instructions.

    Engines execute and complete their own instructions in program order
    (PE matmuls are pc-monotone in start and end; ACT/DVE/Pool are strict
    FIFO with per-op drains), so a wait on the engine's own completion
    semaphore is redundant whenever the instruction carries another wait —
    and walrus's PE/ACT instruction structs only encode a single wait.
    """
    n = 0
    for inst in nc.inst_map.values():
        si = getattr(inst, "sync_info", None)
        if si is None or not si.on_wait or len(si.on_wait) <= 1:
            continue
        eng = str(getattr(inst, "engine", "")).split(".")[-1]
        pref = ENGINE_SEM_PREFIX.get(eng)
        if pref is None:
            continue
        keep = [w for w in si.on_wait if not w.ant_name.startswith(pref)]
        if len(keep) != len(si.on_wait) and keep:
            inst.sync_info = mybir.SyncInfo(
                on_wait=keep, on_update=list(si.on_update or [])
            )
            n += 1
    return n


def build_nc():
    _install_drain_split()
    nc = bass.Bass(trn_type="TRN2", debug=False, num_devices=8)
    # pre-register the exp bias as a const AP (like the 0.0/1.0 preamble
    # consts) so the exps' float bias arg resolves without any dep/wait
    _bt = nc.alloc_sbuf_tensor("const-float32-expbias", [128, 1], F32)
    nc.gpsimd.memset(_bt.ap(), EXP_BIAS)
    nc.const_aps.aps[(F32, EXP_BIAS)] = _bt.ap()
    nc.all_engine_barrier()
    x_d = nc.dram_tensor("x", [C, S], F32, kind="ExternalInput")
    x8_d = nc.dram_tensor("x8", [C, S], F8, kind="ExternalInput")
    w8_d = nc.dram_tensor("w8", [C, 4 * C], F8, kind="ExternalInput")
    bo_d = nc.dram_tensor("bo", [C, 1 + P], F32, kind="ExternalInput")
    y_d = nc.dram_tensor("y", [C, S], F32, kind="ExternalOutput")
    with tile.TileContext(nc) as tc, ExitStack() as ctx:
        trace_kernel(ctx, tc, nc, x_d.ap(), x8_d.ap(), w8_d.ap(), bo_d.ap(),
                     y_d.ap())
    _strip_self_waits(nc)
    if not nc.is_finalized():
        nc.finalize()
    return nc


def host_inputs(x, Wqkv, Wo, bo):
    """Host-side reshard: per-core input dicts (weights replicated)."""
    x = np.ascontiguousarray(np.asarray(x, dtype=np.float32))
    Wqkv = np.asarray(Wqkv, dtype=np.float32)
    Wo = np.asarray(Wo, dtype=np.float32)
    bo = np.asarray(bo, dtype=np.float32)

    # Wqkv rows per head h: [h*3D, h*3D+D) = q, [+D, +2D) = k, [+2D, +3D) = v.
    # q,k channel order: per pair -> [q(2p)|q(2p+1)], [k(2p)|k(2p+1)] tiles.
    order = []
    for p in range(NPAIR):
        for h in (2 * p, 2 * p + 1):
            order.extend(range(h * 3 * D, h * 3 * D + D))          # q rows
        for h in (2 * p, 2 * p + 1):
            order.extend(range(h * 3 * D + D, h * 3 * D + 2 * D))  # k rows
    wqkt = Wqkv[order].T                                            # (C, 2C)
    v_order = [h * 3 * D + 2 * D + d for h in range(NH) for d in range(D)]
    wvt = Wqkv[v_order].T                                           # (C, C)
    import ml_dtypes
    w8 = np.ascontiguousarray(
        np.concatenate([wqkt, wvt, Wo.T], axis=1)
    ).astype(ml_dtypes.float8_e4m3)                                 # (C, 4C)
    # col 0: bo per channel; cols 1:129: a 128x128 identity replicated per
    # k-tile (the +x residual rides the P4 accumulation as I @ x)
    bo2 = np.zeros((C, 1 + 128), dtype=np.float32)
    bo2[:, 0] = bo
    bo2[:, 1:] = np.tile(np.eye(128, dtype=np.float32), (4, 1))

    return [
        dict(x=np.ascontiguousarray(x[b].reshape(C, S)),
             x8=np.ascontiguousarray(x[b].reshape(C, S)).astype(
                 ml_dtypes.float8_e4m3),
             w8=w8, bo=bo2)
        for b in range(B)
    ]


_NC_CACHE = []

try:
    # bass_exec HLO does not embed the BIR; bust jax's executable cache so a
    # rebuilt kernel is actually recompiled instead of hitting a stale NEFF.
    import jax as _jax

    _jax.clear_caches()
except Exception:
    pass


def get_nc():
    if not _NC_CACHE:
        _NC_CACHE.append(build_nc())
    return _NC_CACHE[0]


def run(in_maps, **kwargs):
    return run_bass_kernel_spmd(get_nc(), in_maps, core_ids=list(range(B)), **kwargs)


def kernel(x, Wqkv, Wo, bo):
    in_maps = host_inputs(x, Wqkv, Wo, bo)
    r = run(in_maps)
    y = np.stack([r.results[b]["y"].reshape(C, H, W) for b in range(B)])
    return y.astype(np.float32)


if __name__ == "__main__":
    nc = build_nc()
    print("built ok:", len(nc.inst_map), "instructions")
